# revision 62
# baseline (speedup 1.0000x reference)
"""Trainium2 Bass kernel for a ReActNet-style binary BasicBlock.

Reference math per block (twice, with different weights):
    s   = sign(x + b_in)                      # +-1
    c   = conv3x3(s, mean|w| * sign(w))       # binarized conv, pad=1
    y   = x + ALPHA * c                       # residual
    y   = prelu(y + b_mid, a) + b_out

Key facts exploited:
  * matmul inputs are exactly +-1 -> bf16 matmuls are EXACT (integer sums
    accumulated in fp32 PSUM).
  * per-output-channel weight scale factors out:  conv(s, scale*sign(w)) =
    scale .* conv(s, sign(w)).
  * residual rides through PSUM via an fp32 matmul with diag(1/(ALPHA*scale)):
        T = binconv(s) + x / as           (as = ALPHA*scale, per channel)
    then prelu(x + as*binconv + b, a) = Prelu-activation(T) with
    per-partition scale=as, bias=b, alpha=a  -- a single ScalarE op reading
    PSUM directly.  (prelu positive homogeneity: as > 0.)

Layout: NCHW, channels (64) on partitions; 2 images stacked per 128
partitions (img even -> partitions 0-63, img odd -> 64-127).  Conv matmuls
run as two concurrent 64x64 PE tiles on the array diagonal.  Spatial strips
of R=16 output rows, width padded to 114 with zero columns held in the
sign tiles; conv taps are constant flat-offset shifts.

Sharding: pure data parallel, batch 32 -> 4 images x 8 cores, weights
replicated, no collectives.
"""

import numpy as np
from contextlib import ExitStack

import concourse.bass as bass
import concourse.tile as tile
from concourse import mybir
from concourse import bacc
from concourse.bass_utils import run_bass_kernel_spmd
from concourse.masks import make_identity

B, C, H, W = 32, 64, 112, 112
ALPHA = 0.25
NCORES = 8
BL = B // NCORES          # images per core
WP = W + 2                # padded width
R = 16                    # output rows per strip
NSTRIPS = H // R

F32 = mybir.dt.float32
F32R = mybir.dt.float32r
BF16 = mybir.dt.bfloat16
FP8 = mybir.dt.float8e4

WVEC_NAMES = ["b11", "b12", "b13", "b21", "b22", "b23", "a1", "a2"]

CONV_SCHEME = "ilvp"  # "pair" | "ilvp" | "ilv" | "quad" | "diag2" | "orig"
PIPELINE = True  # emit conv1 of strip s+1 before conv2 of strip s so the
                 # PE stream never waits on the ACT sign2/eviction chain
RES_ON_DVE = False  # inject residual via DVE at eviction instead of a PE matmul
                    # (measured 347us vs 271us for the fp32r matmul -- keep False)


def _bcast_ap(dram_ap, reps=2):
    """Source AP replicating a DRAM tensor across partition groups."""
    return bass.AP(
        tensor=dram_ap.tensor,
        offset=dram_ap.offset,
        ap=[[0, reps]] + [list(d) for d in dram_ap.ap],
    )


def _row_chunks(lo, hi, step=4):
    r = lo
    while r < hi:
        yield r, min(step, hi - r)
        r += step


def build_program(bl=BL, loop_n=None):
    """Build the Bass program for one core processing `bl` images.

    loop_n: if set, repeat the whole main loop on-device that many times
    (timing harness only -- results identical, just recomputed).
    """
    nc = bacc.Bacc("TRN2", target_bir_lowering=False, debug=False)

    x_d = nc.dram_tensor("x", [bl, C, H, W], F32, kind="ExternalInput").ap()
    w3_d = nc.dram_tensor("w3", [C, C, 3, 3], F32, kind="ExternalInput").ap()
    wpw_d = nc.dram_tensor("w_pw", [C, C, 3, 3], F32, kind="ExternalInput").ap()
    vec_d = {
        n: nc.dram_tensor(n, [C], F32, kind="ExternalInput").ap()
        for n in WVEC_NAMES
    }
    out_d = nc.dram_tensor("out", [bl, C, H, W], F32, kind="ExternalOutput").ap()

    with tile.TileContext(nc) as tc:
        _kernel_body(tc, out_d, x_d, w3_d, wpw_d, vec_d, bl, loop_n=loop_n)

    nc.compile()
    return nc


def _prep_conv_consts(nc, const, wdram, name, pair=False):
    """Per-conv constants: binarized-transposed weights, as=ALPHA*mean|w|,
    diag(1/as) for the residual matmul.

    pair=False: weights replicated identically on both partition halves.
    pair=True: partitions 64-127 hold the taps shifted by one flat element
    (halfB[p, co, t] = halfA[p, co, t+1]) so a K=128 matmul against a
    [sign; sign-shifted-by-1-col] data tile computes two taps at once.
    """
    # natural layout [co, ci*3*3] duplicated -> per-channel scale
    wn = const.tile([128, C * 9], F32, name=f"wn_{name}")
    nc.sync.dma_start(out=wn, in_=_bcast_ap(wdram.rearrange("a b c d -> a (b c d)")))
    wabs = const.tile([128, C * 9], F32, name=f"wabs_{name}")
    asum = const.tile([128, 1], F32, name=f"asum_{name}")
    nc.scalar.activation(
        out=wabs, in_=wn, func=mybir.ActivationFunctionType.Abs, accum_out=asum
    )
    asc = const.tile([128, 1], F32, name=f"asc_{name}")  # ALPHA * mean|w|
    nc.vector.tensor_scalar_mul(asc, asum, ALPHA / (C * 9))
    inv_asc = const.tile([128, 1], F32, name=f"iasc_{name}")
    nc.vector.reciprocal(inv_asc, asc)

    # gathered+transposed weights [ci(+dup), co, tap], then binarize to bf16
    wg = const.tile([128, C, 9], F32, name=f"wg_{name}")
    src = bass.AP(
        tensor=wdram.tensor,
        offset=wdram.offset,
        ap=[[9, C], [C * 9, C], [1, 9]],
    )
    nc.sync.dma_start(out=wg[0:64, :, :], in_=src)
    if pair is True:
        # halfB = taps shifted by one flat element (tap t -> t+1); tap 8
        # of halfB is never consumed by any matmul, fill with 1.0
        src1 = bass.AP(
            tensor=wdram.tensor, offset=wdram.offset + 1,
            ap=[[9, C], [C * 9, C], [1, 8]],
        )
        nc.sync.dma_start(out=wg[64:128, :, 0:8], in_=src1)
        nc.gpsimd.memset(wg[64:128, :, 8:9], 1.0)
    else:
        nc.sync.dma_start(out=wg[64:128, :, :], in_=src)
    wsign = const.tile([128, C, 9], BF16, name=f"ws_{name}")
    nc.scalar.activation(out=wsign, in_=wg, func=mybir.ActivationFunctionType.Sign)

    if pair == "dr":
        # fp8 weights for DoubleRow: ws8 [ci(+dup), co, tap] single-tap
        # weights; wdr [ci(+dup), pairslot, ko, co] with ko = the two taps
        # (3r, 3r+1) packed per PE cell (2 fp8 weights/cell, 0.5 cyc/row).
        ws8 = const.tile([128, C, 9], FP8, name=f"w8_{name}")
        nc.scalar.activation(
            out=ws8, in_=wg, func=mybir.ActivationFunctionType.Sign
        )
        # block-diagonal in the output dim: one full-width DoubleRow matmul
        # computes the tap pair for BOTH images (DR requires dst partition
        # base 0, so the quadrant form is not codegen-able)
        wdr = const.tile([128, 3, 2, 128], FP8, name=f"wdr_{name}")
        nc.vector.memset(wdr, 0.0)
        for i in range(3):
            for k in range(2):
                for half in range(2):
                    psl = slice(64 * half, 64 * half + 64)
                    nc.vector.tensor_copy(
                        out=wdr[psl, i, k, 64 * half : 64 * half + 64],
                        in_=ws8[psl, :, 3 * i + k],
                    )
        wsign = (ws8, wdr)
    if pair == "bdg":
        # block-diagonal per-tap weights [ci(2 imgs), tap, co(2 imgs)]:
        # wblk[k, t, m] = sign(w)[m % 64, k % 64, t] when (k < 64) == (m < 64)
        # else 0, so ONE K=128 matmul per tap computes both images (imgA ->
        # psum 0-63, imgB -> 64-127) with no reliance on PE tile overlap.
        wblk = const.tile([128, 9, 128], BF16, name=f"wb_{name}")
        nc.vector.memset(wblk, 0.0)
        for half in range(2):
            psl = slice(64 * half, 64 * half + 64)
            nc.vector.tensor_copy(
                out=wblk[psl, :, 64 * half : 64 * half + 64].rearrange(
                    "p t c -> p c t"
                ),
                in_=wsign[psl, :, :],
            )
        wsign = wblk

    # residual injector: full 128x128 diag(1/as), fp32r so the matmul runs
    # at bf16 rate (1 cycle/row at n>=256 vs 4 for plain fp32) and covers
    # BOTH image halves in a single matmul (fp32r requires dst partition
    # base 0).  The BIR verifier requires every producer of fp32r-matmul
    # operands to emit fp32r-rounded output, so the scaled identity is
    # materialized via a DVE op with an fp32r-typed destination.
    ident = const.tile([128, 128], F32, name=f"id_{name}")
    make_identity(nc, ident)
    identr = const.tile([128, 128], F32R, name=f"idr_{name}")
    nc.vector.tensor_scalar_mul(identr, ident, inv_asc)
    return wsign, asc, identr


def _conv_bankset(nc, pt, w, ident, s_t, soff, res_t, roff, n, asc, bias_mid,
                  alpha, out_ap, ev):
    """One 2-bank PSUM set covering both images of the pair.

    Four concurrent 64x64 PE quadrants:
      (row 0, col 0): imgA subset1 -> bank0[0:64]
      (row 64, col 64): imgB subset1 -> bank0[64:128]
      (row 0, col 64): imgA subset2 -> bank1[64:128]   (crossed)
      (row 64, col 0): imgB subset2 -> bank1[0:64]     (crossed)
    subset1 = {residual-injector matmul, taps 0-3}; subset2 = {taps 4-8}.
    Eviction: u = as*bank0 + bias (ACT, full width); u += as*bank1 via two
    half-width cross-partition STTs (DVE); out = Prelu(u) (ACT).
    """
    lo, hi = slice(0, 64), slice(64, 128)
    if CONV_SCHEME == "ilv":
        # like orig, but the two diagonal quadrants' matmuls are interleaved
        # per tap slot: MMs start in strict program order, so lower/upper
        # pairs must be adjacent in the stream to actually run concurrently
        for s in range(10):
            for rsl in (lo, hi):
                if s < 9:
                    nc.tensor.matmul(
                        pt[rsl, :n], w[rsl, :, s],
                        s_t[rsl, soff(s) : soff(s) + n],
                        start=(s == 0), stop=False, skip_group_check=True,
                    )
                else:
                    nc.tensor.matmul(
                        pt[rsl, :n], ident[rsl, :], res_t[rsl, roff : roff + n],
                        start=False, stop=True, skip_group_check=True,
                    )
        nc.scalar.activation(
            out=out_ap, in_=pt[:, :n],
            func=mybir.ActivationFunctionType.Prelu,
            bias=bias_mid, scale=asc, alpha=alpha,
        )
        return
    if CONV_SCHEME == "orig":
        # single-bank psum, serial taps per half, Prelu direct from PSUM
        for rsl in (lo, hi):
            for t in range(9):
                nc.tensor.matmul(
                    pt[rsl, :n], w[rsl, :, t], s_t[rsl, soff(t) : soff(t) + n],
                    start=(t == 0), stop=False, skip_group_check=True,
                )
            nc.tensor.matmul(
                pt[rsl, :n], ident[rsl, :], res_t[rsl, roff : roff + n],
                start=False, stop=True, skip_group_check=True,
            )
        nc.scalar.activation(
            out=out_ap, in_=pt[:, :n],
            func=mybir.ActivationFunctionType.Prelu,
            bias=bias_mid, scale=asc, alpha=alpha,
        )
        return
    if CONV_SCHEME == "quad":
        quads = [(lo, lo, 0), (hi, hi, 0), (lo, hi, 1), (hi, lo, 1)]
    else:  # diag2: only diagonal quadrants, banks aligned
        quads = [(lo, lo, 0), (hi, hi, 0), (lo, lo, 1), (hi, hi, 1)]
    slots = (
        [("id", None), ("tap", 0), ("tap", 1), ("tap", 2), ("tap", 3)],
        [("tap", 4), ("tap", 5), ("tap", 6), ("tap", 7), ("tap", 8)],
    )
    for s in range(5):
        for rsl, osl, sub in quads:
            kind, t = slots[sub][s]
            boff = 0 if sub == 0 else 512
            out = pt[osl, boff : boff + n]
            if kind == "id":
                nc.tensor.matmul(
                    out, ident[rsl, :], res_t[rsl, roff : roff + n],
                    start=True, stop=False, skip_group_check=True,
                )
            else:
                nc.tensor.matmul(
                    out, w[rsl, :, t], s_t[rsl, soff(t) : soff(t) + n],
                    start=(s == 0), stop=(s == 4), skip_group_check=True,
                )
    nc.scalar.activation(
        out=ev[:, :n], in_=pt[:, 0:n],
        func=mybir.ActivationFunctionType.Identity, bias=bias_mid, scale=asc,
    )
    if CONV_SCHEME == "quad":
        nc.vector.scalar_tensor_tensor(
            out=ev[0:64, :n], in0=pt[64:128, 512 : 512 + n], scalar=asc[0:64],
            in1=ev[0:64, :n], op0=mybir.AluOpType.mult, op1=mybir.AluOpType.add,
        )
        nc.vector.scalar_tensor_tensor(
            out=ev[64:128, :n], in0=pt[0:64, 512 : 512 + n],
            scalar=asc[64:128], in1=ev[64:128, :n],
            op0=mybir.AluOpType.mult, op1=mybir.AluOpType.add,
        )
    else:
        nc.vector.scalar_tensor_tensor(
            out=ev[:, :n], in0=pt[:, 512 : 512 + n], scalar=asc,
            in1=ev[:, :n], op0=mybir.AluOpType.mult, op1=mybir.AluOpType.add,
        )
    nc.scalar.activation(
        out=out_ap, in_=ev[:, :n],
        func=mybir.ActivationFunctionType.Prelu, alpha=alpha,
    )




def _conv_group(nc, pts, w, ident, s_t, soffs, res_t, roffs, ns, asc,
                bias_mid, alpha, out_aps, evs=None):
    """Bank-paired taps-outer conv: each weight load feeds the matmuls of
    all banks in the group before the next load (halves LDWEIGHTS rate);
    lower/upper quadrant pairs stay adjacent for concurrency.

    Residual injection: either a trailing fp32r identity matmul into PSUM
    (RES_ON_DVE=False) or a DVE scalar_tensor_tensor at eviction
    (ev = as*psum + res; PE stream shrinks to 9 homogeneous tap slots)."""
    lo, hi = slice(0, 64), slice(64, 128)
    for s in range(9):
        for rsl in (lo, hi):
            for pt, n, soff, roff in zip(pts, ns, soffs, roffs):
                nc.tensor.matmul(
                    pt[rsl, :n], w[rsl, :, s],
                    s_t[rsl, soff(s) : soff(s) + n],
                    start=(s == 0), stop=(RES_ON_DVE and s == 8),
                    skip_group_check=True,
                )
    if RES_ON_DVE:
        for pt, n, roff, oap, ev in zip(pts, ns, roffs, out_aps, evs):
            nc.vector.scalar_tensor_tensor(
                out=ev[:, :n], in0=pt[:, :n], scalar=asc,
                in1=res_t[:, roff : roff + n],
                op0=mybir.AluOpType.mult, op1=mybir.AluOpType.add,
            )
            nc.scalar.activation(
                out=oap, in_=ev[:, :n],
                func=mybir.ActivationFunctionType.Prelu,
                bias=bias_mid, alpha=alpha,
            )
        return
    for pt, n, soff, roff in zip(pts, ns, soffs, roffs):
        nc.tensor.matmul(
            pt[:, :n], ident,
            res_t[:, roff : roff + n].bitcast(F32R),
            start=False, stop=True, skip_group_check=True,
        )
    for pt, n, oap in zip(pts, ns, out_aps):
        nc.scalar.activation(
            out=oap, in_=pt[:, :n],
            func=mybir.ActivationFunctionType.Prelu,
            bias=bias_mid, scale=asc, alpha=alpha,
        )

def _sp_build(nc, sppool, ss_t, L, tag):
    """Pairing tile [128, 2L]: per-image segment of L columns; partitions
    0-63 = that image's sign data, 64-127 = same shifted left by one
    element, so a K=128 matmul reading offset o accumulates taps o and
    o+1 in one pass."""
    sp_t = sppool.tile([128, 2 * L], BF16, tag=tag)
    nc.vector.tensor_copy(out=sp_t[0:64, 0:L], in_=ss_t[0:64, 0:L])
    nc.vector.tensor_copy(out=sp_t[64:128, 0 : L - 1], in_=ss_t[0:64, 1:L])
    nc.vector.tensor_copy(out=sp_t[0:64, L : 2 * L], in_=ss_t[64:128, 0:L])
    nc.vector.tensor_copy(
        out=sp_t[64:128, L : 2 * L - 1], in_=ss_t[64:128, 1:L]
    )
    return sp_t


def _conv_group_pair(nc, pts, w, ident, ss_t, sp_t, seg, soffs, res_t, roffs,
                     ns, asc, bias_mid, alpha, out_aps):
    """Tap-paired conv: 3 K=128 pair-matmuls (taps (r,-1)+(r,0) via the
    shifted-dup data tile, one per kernel row), 3 K=64 single-tap matmuls
    (taps (r,+1), quadrant-concurrent per image), 1 full-width fp32r
    residual matmul.  7 column-slots/bankset vs 10 in the ilvp scheme.
    Images ride the PE column dimension: imgA -> PSUM 0-63, imgB -> 64-127.
    """
    lo, hi = slice(0, 64), slice(64, 128)
    for i, tl in enumerate((0, 3, 6)):  # left tap of each pair (r, -1)
        for pt, n, soff in zip(pts, ns, soffs):
            for img, osl in ((0, lo), (1, hi)):
                nc.tensor.matmul(
                    pt[osl, :n], w[:, :, tl],
                    sp_t[:, img * seg + soff(tl) : img * seg + soff(tl) + n],
                    start=(i == 0), stop=False, skip_group_check=True,
                )
    for tl in (2, 5, 8):  # leftover taps (r, +1); imgB half holds tap t at
        for pt, n, soff in zip(pts, ns, soffs):  # slot t-1 (shifted layout)
            nc.tensor.matmul(
                pt[lo, :n], w[lo, :, tl],
                ss_t[lo, soff(tl) : soff(tl) + n],
                start=False, stop=False, skip_group_check=True,
            )
            nc.tensor.matmul(
                pt[hi, :n], w[hi, :, tl - 1],
                ss_t[hi, soff(tl) : soff(tl) + n],
                start=False, stop=False, skip_group_check=True,
            )
    for pt, n, roff in zip(pts, ns, roffs):
        nc.tensor.matmul(
            pt[:, :n], ident,
            res_t[:, roff : roff + n].bitcast(F32R),
            start=False, stop=True, skip_group_check=True,
        )
    for pt, n, oap in zip(pts, ns, out_aps):
        nc.scalar.activation(
            out=oap, in_=pt[:, :n],
            func=mybir.ActivationFunctionType.Prelu,
            bias=bias_mid, scale=asc, alpha=alpha,
        )


def _conv_group_dr(nc, pts, w8, wdr, ident, sp_t, soffs, res_t, roffs, ns,
                   asc, bias_mid, alpha, out_aps):
    """fp8 DoubleRow conv: the sign tile holds two planes (straight +
    shifted-by-one-column); each DoubleRow matmul packs taps (3r, 3r+1)
    as 2 fp8 weights per PE cell at 0.5 cycles/row, quadrant-concurrent
    across the two images.  3 DR pair-slots + 3 single-tap fp8 slots +
    1 fp32r residual per bankset."""
    lo, hi = slice(0, 64), slice(64, 128)
    for i in range(3):
        for pt, n, soff in zip(pts, ns, soffs):
            o = soff(3 * i)
            nc.tensor.matmul(
                pt[:, :n], wdr[:, i, :, :],
                sp_t[:, :, o : o + n],
                perf_mode=mybir.MatmulPerfMode.DoubleRow,
                start=(i == 0), stop=False, skip_group_check=True,
            )
    for t in (2, 5, 8):
        for rsl in (lo, hi):
            for pt, n, soff in zip(pts, ns, soffs):
                o = soff(t)
                nc.tensor.matmul(
                    pt[rsl, :n], w8[rsl, :, t],
                    sp_t[rsl, 0, o : o + n],
                    start=False, stop=False, skip_group_check=True,
                )
    for pt, n, roff in zip(pts, ns, roffs):
        nc.tensor.matmul(
            pt[:, :n], ident,
            res_t[:, roff : roff + n].bitcast(F32R),
            start=False, stop=True, skip_group_check=True,
        )
    for pt, n, oap in zip(pts, ns, out_aps):
        nc.scalar.activation(
            out=oap, in_=pt[:, :n],
            func=mybir.ActivationFunctionType.Prelu,
            bias=bias_mid, scale=asc, alpha=alpha,
        )


def _conv_group_bdg(nc, pts, w, ident, s_t, soffs, res_t, roffs, ns, asc,
                    bias_mid, alpha, out_aps):
    """Block-diagonal conv: each tap is ONE K=128 full-width matmul
    covering both images (no PE tile-overlap needed); 9 taps + 1 fp32r
    residual = 10 column-slots per bankset."""
    for t in range(9):
        for pt, n, soff in zip(pts, ns, soffs):
            nc.tensor.matmul(
                pt[:, :n], w[:, t, :],
                s_t[:, soff(t) : soff(t) + n],
                start=(t == 0), stop=False, skip_group_check=True,
            )
    for pt, n, roff in zip(pts, ns, roffs):
        nc.tensor.matmul(
            pt[:, :n], ident,
            res_t[:, roff : roff + n].bitcast(F32R),
            start=False, stop=True, skip_group_check=True,
        )
    for pt, n, oap in zip(pts, ns, out_aps):
        nc.scalar.activation(
            out=oap, in_=pt[:, :n],
            func=mybir.ActivationFunctionType.Prelu,
            bias=bias_mid, scale=asc, alpha=alpha,
        )


def _kernel_body(tc, out_d, x_d, w3_d, wpw_d, vec_d, bl, loop_n=None):
    nc = tc.nc
    ctx = ExitStack()
    with ctx:
        const = ctx.enter_context(tc.tile_pool(name="const", bufs=1))
        xpool = ctx.enter_context(tc.tile_pool(name="xpool", bufs=2))
        s1pool = ctx.enter_context(tc.tile_pool(name="s1pool", bufs=2))
        p1pool = ctx.enter_context(tc.tile_pool(name="p1pool", bufs=2))
        s2pool = ctx.enter_context(tc.tile_pool(name="s2pool", bufs=2))
        p2pool = ctx.enter_context(tc.tile_pool(name="p2pool", bufs=2))
        evpool = ctx.enter_context(tc.tile_pool(name="evpool", bufs=4))
        if CONV_SCHEME == "pair":
            sp1pool = ctx.enter_context(tc.tile_pool(name="sp1pool", bufs=2))
            sp2pool = ctx.enter_context(tc.tile_pool(name="sp2pool", bufs=2))
        else:
            sp1pool = sp2pool = None
        PS_SHAPE = ([128, 456]
                    if CONV_SCHEME in ("orig", "ilv", "ilvp", "pair", "bdg",
                                       "dr")
                    else [128, 1024])
        PS_BUFS = (4 if CONV_SCHEME in ("ilvp", "pair", "bdg", "dr")
                   else (3 if CONV_SCHEME in ("orig", "ilv") else 2))
        ps1 = ctx.enter_context(tc.tile_pool(name="ps1", bufs=PS_BUFS, space="PSUM"))
        ps2 = ctx.enter_context(tc.tile_pool(name="ps2", bufs=PS_BUFS, space="PSUM"))

        # ---- constants -------------------------------------------------
        v = {}
        for n in WVEC_NAMES:
            v[n] = const.tile([128, 1], F32, name=f"v_{n}")
            nc.sync.dma_start(out=v[n], in_=_bcast_ap(vec_d[n]))
        b31 = const.tile([128, 1], F32, name="b31")  # b13 + b21
        nc.vector.tensor_tensor(
            out=b31, in0=v["b13"], in1=v["b21"], op=mybir.AluOpType.add
        )
        b32 = const.tile([128, 1], F32, name="b32")  # b13 + b22
        nc.vector.tensor_tensor(
            out=b32, in0=v["b13"], in1=v["b22"], op=mybir.AluOpType.add
        )

        pair = (CONV_SCHEME if CONV_SCHEME in ("bdg", "dr")
                else (CONV_SCHEME == "pair"))
        w1, as1, id1 = _prep_conv_consts(nc, const, w3_d, "c1", pair=pair)
        w2, as2, id2 = _prep_conv_consts(nc, const, wpw_d, "c2", pair=pair)

        # ---- main loop -------------------------------------------------
        X_ROWS = R + 4     # x / s1 strip rows   [h0-2, h0+R+2)
        P_ROWS = R + 2     # p1 / s2 strip rows  [h0-1, h0+R+1)
        X_LEN = X_ROWS * WP
        P_LEN = P_ROWS * WP

        from contextlib import nullcontext
        loop_cm = tc.For_i(0, loop_n, 1) if loop_n else nullcontext()
        with loop_cm:
            if PIPELINE and CONV_SCHEME == "ilvp" and not RES_ON_DVE:
                _main_strips_pipe(tc, nc, out_d, x_d, bl, v, b31, b32,
                                  w1, as1, id1, w2, as2, id2,
                                  xpool, s1pool, p1pool, s2pool, p2pool,
                                  evpool, ps1, ps2, PS_SHAPE)
            else:
                _main_strips(tc, nc, out_d, x_d, bl, v, b31, b32,
                             w1, as1, id1, w2, as2, id2,
                             xpool, s1pool, p1pool, s2pool, p2pool, evpool,
                             ps1, ps2, PS_SHAPE, sp1pool, sp2pool)


def _main_strips_pipe(tc, nc, out_d, x_d, bl, v, b31, b32, w1, as1, id1,
                      w2, as2, id2, xpool, s1pool, p1pool, s2pool, p2pool,
                      evpool, ps1, ps2, PS_SHAPE):
    """Software-pipelined strip loop (ilvp + fp32r residual only): the
    conv1 stage of iteration i+1 is emitted before the conv2 stage of
    iteration i, so the PE always has the next strip's tap matmuls queued
    while ACT works through sign2/evictions of the current one."""
    X_ROWS = R + 4
    P_ROWS = R + 2
    X_LEN = X_ROWS * WP
    P_LEN = P_ROWS * WP

    def front(imgs, h0):
        xlo, xhi = max(h0 - 2, 0), min(h0 + R + 2, H)
        c1lo, c1hi = max(h0 - 1, 0), min(h0 + R + 1, H)

        def xloc(g):
            return g - (h0 - 2)

        def ploc(g):
            return g - (h0 - 1)

        x_t = xpool.tile([128, X_LEN + 4], F32, tag="x")
        x_r = x_t[:, 2 : 2 + X_LEN].rearrange("p (r c) -> p r c", c=WP)
        for j in range(2):
            nc.sync.dma_start(
                out=x_r[
                    64 * j : 64 * j + 64, xloc(xlo) : xloc(xhi), 1 : 1 + W
                ].bitcast(F32R),
                in_=x_d[imgs[j], :, xlo:xhi, :].bitcast(F32R),
            )
        for pc in (0, WP - 1):
            nc.scalar.activation(
                out=x_r[:, xloc(xlo) : xloc(xhi), pc : pc + 1].bitcast(F32R),
                in_=x_r[:, xloc(xlo) : xloc(xhi), 1:2],
                func=mybir.ActivationFunctionType.Copy,
                scale=0.0,
            )

        s1_t = s1pool.tile([128, X_LEN + 4], BF16, tag="s1")
        s1_r = s1_t[:, 2 : 2 + X_LEN].rearrange("p (r c) -> p r c", c=WP)
        nc.scalar.activation(
            out=s1_t[:, 2 + xloc(xlo) * WP : 2 + xloc(xhi) * WP],
            in_=x_t[:, 2 + xloc(xlo) * WP : 2 + xloc(xhi) * WP],
            func=mybir.ActivationFunctionType.Sign,
            bias=v["b11"],
        )
        nc.gpsimd.memset(s1_r[:, :, 0:1], 0.0)
        nc.gpsimd.memset(s1_r[:, :, WP - 1 : WP], 0.0)
        nc.gpsimd.memset(s1_t[:, 0:2], 0.0)
        nc.gpsimd.memset(s1_t[:, 2 + X_LEN :], 0.0)
        if xloc(xlo) > 0:
            nc.gpsimd.memset(s1_t[:, 2 : 2 + xloc(xlo) * WP], 0.0)
        if xloc(xhi) < X_ROWS:
            nc.gpsimd.memset(s1_t[:, 2 + xloc(xhi) * WP : 2 + X_LEN], 0.0)

        p1_t = p1pool.tile([128, P_LEN + 4], F32R, tag="p1")
        chunks = list(_row_chunks(c1lo, c1hi))
        for i in range(0, len(chunks), 2):
            grp = chunks[i : i + 2]
            _conv_group(
                nc,
                [ps1.tile(PS_SHAPE, F32, tag="ps1", name="pt1") for _ in grp],
                w1, id1, s1_t,
                soffs=[
                    lambda t, _r=r0: 2
                    + (xloc(_r) + t // 3 - 1) * WP + (t % 3 - 1)
                    for r0, _ in grp
                ],
                res_t=x_t,
                roffs=[2 + xloc(r0) * WP for r0, _ in grp],
                ns=[nr * WP for _, nr in grp],
                asc=as1, bias_mid=v["b12"], alpha=v["a1"],
                out_aps=[
                    p1_t[:, 2 + ploc(r0) * WP : 2 + (ploc(r0) + nr) * WP]
                    for r0, nr in grp
                ],
            )
        return imgs, h0, ploc, p1_t, c1lo, c1hi

    def back(st):
        imgs, h0, ploc, p1_t, c1lo, c1hi = st
        s2_t = s2pool.tile([128, P_LEN + 4], BF16, tag="s2")
        s2_r = s2_t[:, 2 : 2 + P_LEN].rearrange("p (r c) -> p r c", c=WP)
        nc.scalar.activation(
            out=s2_t[:, 2 + ploc(c1lo) * WP : 2 + ploc(c1hi) * WP],
            in_=p1_t[:, 2 + ploc(c1lo) * WP : 2 + ploc(c1hi) * WP].bitcast(F32),
            func=mybir.ActivationFunctionType.Sign,
            bias=b31,
        )
        nc.gpsimd.memset(s2_r[:, :, 0:1], 0.0)
        nc.gpsimd.memset(s2_r[:, :, WP - 1 : WP], 0.0)
        nc.gpsimd.memset(s2_t[:, 0:2], 0.0)
        nc.gpsimd.memset(s2_t[:, 2 + P_LEN :], 0.0)
        if ploc(c1lo) > 0:
            nc.gpsimd.memset(s2_t[:, 2 : 2 + ploc(c1lo) * WP], 0.0)
        if ploc(c1hi) < P_ROWS:
            nc.gpsimd.memset(s2_t[:, 2 + ploc(c1hi) * WP : 2 + P_LEN], 0.0)

        p2_t = p2pool.tile([128, R * WP], F32, tag="p2")
        chunks = list(_row_chunks(h0, h0 + R))
        for i in range(0, len(chunks), 2):
            grp = chunks[i : i + 2]
            _conv_group(
                nc,
                [ps2.tile(PS_SHAPE, F32, tag="ps2", name="pt2") for _ in grp],
                w2, id2, s2_t,
                soffs=[
                    lambda t, _r=r0: 2
                    + (ploc(_r) + t // 3 - 1) * WP + (t % 3 - 1)
                    for r0, _ in grp
                ],
                res_t=p1_t,
                roffs=[2 + ploc(r0) * WP for r0, _ in grp],
                ns=[nr * WP for _, nr in grp],
                asc=as2, bias_mid=b32, alpha=v["a2"],
                out_aps=[
                    p2_t[:, (r0 - h0) * WP : (r0 - h0 + nr) * WP]
                    for r0, nr in grp
                ],
            )
        nc.vector.tensor_scalar_add(p2_t, p2_t, v["b23"])
        p2_r = p2_t.rearrange("p (r c) -> p r c", c=WP)
        for j in range(2):
            nc.sync.dma_start(
                out=out_d[imgs[j], :, h0 : h0 + R, :],
                in_=p2_r[64 * j : 64 * j + 64, :, 1 : 1 + W],
            )

    items = [((2 * p, 2 * p + 1), s * R)
             for p in range(bl // 2) for s in range(NSTRIPS)]
    prev = None
    for it in items:
        st = front(*it)
        if prev is not None:
            back(prev)
        prev = st
    back(prev)


def _main_strips(tc, nc, out_d, x_d, bl, v, b31, b32, w1, as1, id1,
                 w2, as2, id2, xpool, s1pool, p1pool, s2pool, p2pool,
                 evpool, ps1, ps2, PS_SHAPE, sp1pool=None, sp2pool=None):
        X_ROWS = R + 4
        P_ROWS = R + 2
        X_LEN = X_ROWS * WP
        P_LEN = P_ROWS * WP
        for pair in range(bl // 2):
            imgs = (2 * pair, 2 * pair + 1)
            for s in range(NSTRIPS):
                h0 = s * R
                xlo, xhi = max(h0 - 2, 0), min(h0 + R + 2, H)
                c1lo, c1hi = max(h0 - 1, 0), min(h0 + R + 1, H)

                def xloc(g):   # global row -> local row in x/s1 strip
                    return g - (h0 - 2)

                def ploc(g):   # global row -> local row in p1/s2 strip
                    return g - (h0 - 1)

                # -- load x ---------------------------------------------
                x_t = xpool.tile([128, X_LEN + 4], F32, tag="x")
                x_r = x_t[:, 2 : 2 + X_LEN].rearrange(
                    "p (r c) -> p r c", c=WP
                )
                # With the fp32r residual matmul (RES_ON_DVE=False), all
                # writers go through fp32r-typed APs so the matmul operand
                # passes BIR verification (bits are plain fp32 either way);
                # Pool memset can't emit fp32r, so pads are zeroed by ACT
                # scale-0 copies instead.
                xr_cast = (lambda ap: ap) if RES_ON_DVE else (
                    lambda ap: ap.bitcast(F32R))
                for j in range(2):
                    nc.sync.dma_start(
                        out=xr_cast(x_r[
                            64 * j : 64 * j + 64,
                            xloc(xlo) : xloc(xhi),
                            1 : 1 + W,
                        ]),
                        in_=xr_cast(x_d[imgs[j], :, xlo:xhi, :]),
                    )
                if RES_ON_DVE:
                    nc.gpsimd.memset(x_r[:, :, 0:1], 0.0)
                    nc.gpsimd.memset(x_r[:, :, WP - 1 : WP], 0.0)
                else:
                    for pc in (0, WP - 1):
                        nc.scalar.activation(
                            out=x_r[:, xloc(xlo) : xloc(xhi), pc : pc + 1].bitcast(F32R),
                            in_=x_r[:, xloc(xlo) : xloc(xhi), 1:2],
                            func=mybir.ActivationFunctionType.Copy,
                            scale=0.0,
                        )

                # -- s1 = sign(x + b11), zero padding -------------------
                if CONV_SCHEME == "dr":
                    SEG1 = -(-(X_LEN + 4) // 16) * 16
                    s1p_t = s1pool.tile([128, 2, SEG1], FP8, tag="s1")
                    s1_t = s1p_t[:, 0, :]  # plane 0: straight sign data
                else:
                    s1_t = s1pool.tile([128, X_LEN + 4], BF16, tag="s1")
                s1_r = s1_t[:, 2 : 2 + X_LEN].rearrange(
                    "p (r c) -> p r c", c=WP
                )
                nc.scalar.activation(
                    out=s1_t[:, 2 + xloc(xlo) * WP : 2 + xloc(xhi) * WP],
                    in_=x_t[:, 2 + xloc(xlo) * WP : 2 + xloc(xhi) * WP],
                    func=mybir.ActivationFunctionType.Sign,
                    bias=v["b11"],
                )
                nc.gpsimd.memset(s1_r[:, :, 0:1], 0.0)
                nc.gpsimd.memset(s1_r[:, :, WP - 1 : WP], 0.0)
                nc.gpsimd.memset(s1_t[:, 0:2], 0.0)
                nc.gpsimd.memset(s1_t[:, 2 + X_LEN :], 0.0)
                if xloc(xlo) > 0:  # top image edge
                    nc.gpsimd.memset(s1_t[:, 2 : 2 + xloc(xlo) * WP], 0.0)
                if xloc(xhi) < X_ROWS:  # bottom image edge
                    nc.gpsimd.memset(
                        s1_t[:, 2 + xloc(xhi) * WP : 2 + X_LEN], 0.0
                    )
                if CONV_SCHEME == "dr":
                    # plane 1 = plane 0 shifted left one column, so a
                    # DoubleRow matmul reading [plane0; plane1] at offset
                    # o accumulates taps o and o+1 in one pass
                    nc.vector.tensor_copy(
                        out=s1p_t[:, 1, 0 : SEG1 - 1],
                        in_=s1p_t[:, 0, 1:SEG1],
                    )
                    nc.gpsimd.memset(s1p_t[:, 1, SEG1 - 1 : SEG1], 0.0)

                # -- conv1 + fused residual/scale/bias/prelu ------------
                # fp32r-typed (unless RES_ON_DVE): conv2's residual matmul
                # consumes it, so its producer (the conv1 eviction ACT)
                # must round to fp32r
                p1_t = p1pool.tile([128, P_LEN + 4],
                                   F32 if RES_ON_DVE else F32R, tag="p1")
                if CONV_SCHEME == "pair":
                    sp1_t = _sp_build(nc, sp1pool, s1_t, X_LEN + 4, "sp1")
                    chunks = list(_row_chunks(c1lo, c1hi))
                    for i in range(0, len(chunks), 2):
                        grp = chunks[i : i + 2]
                        _conv_group_pair(
                            nc,
                            [ps1.tile(PS_SHAPE, F32, tag="ps1", name="pt1")
                             for _ in grp],
                            w1, id1, s1_t, sp1_t, X_LEN + 4,
                            soffs=[
                                lambda t, _r=r0: 2
                                + (xloc(_r) + t // 3 - 1) * WP + (t % 3 - 1)
                                for r0, _ in grp
                            ],
                            res_t=x_t,
                            roffs=[2 + xloc(r0) * WP for r0, _ in grp],
                            ns=[nr * WP for _, nr in grp],
                            asc=as1, bias_mid=v["b12"], alpha=v["a1"],
                            out_aps=[
                                p1_t[:, 2 + ploc(r0) * WP : 2 + (ploc(r0) + nr) * WP]
                                for r0, nr in grp
                            ],
                        )
                if CONV_SCHEME == "dr":
                    chunks = list(_row_chunks(c1lo, c1hi))
                    for i in range(0, len(chunks), 2):
                        grp = chunks[i : i + 2]
                        _conv_group_dr(
                            nc,
                            [ps1.tile(PS_SHAPE, F32, tag="ps1", name="pt1")
                             for _ in grp],
                            w1[0], w1[1], id1, s1p_t,
                            soffs=[
                                lambda t, _r=r0: 2
                                + (xloc(_r) + t // 3 - 1) * WP + (t % 3 - 1)
                                for r0, _ in grp
                            ],
                            res_t=x_t,
                            roffs=[2 + xloc(r0) * WP for r0, _ in grp],
                            ns=[nr * WP for _, nr in grp],
                            asc=as1, bias_mid=v["b12"], alpha=v["a1"],
                            out_aps=[
                                p1_t[:, 2 + ploc(r0) * WP : 2 + (ploc(r0) + nr) * WP]
                                for r0, nr in grp
                            ],
                        )
                if CONV_SCHEME in ("ilvp", "bdg"):
                    chunks = list(_row_chunks(c1lo, c1hi))
                    grp_fn = (_conv_group if CONV_SCHEME == "ilvp"
                              else _conv_group_bdg)
                    for i in range(0, len(chunks), 2):
                        grp = chunks[i : i + 2]
                        grp_fn(
                            nc,
                            [ps1.tile(PS_SHAPE, F32, tag="ps1", name="pt1")
                             for _ in grp],
                            w1, id1, s1_t,
                            soffs=[
                                lambda t, _r=r0: 2
                                + (xloc(_r) + t // 3 - 1) * WP + (t % 3 - 1)
                                for r0, _ in grp
                            ],
                            res_t=x_t,
                            roffs=[2 + xloc(r0) * WP for r0, _ in grp],
                            ns=[nr * WP for _, nr in grp],
                            asc=as1, bias_mid=v["b12"], alpha=v["a1"],
                            out_aps=[
                                p1_t[:, 2 + ploc(r0) * WP : 2 + (ploc(r0) + nr) * WP]
                                for r0, nr in grp
                            ],
                            **({"evs": [evpool.tile([128, 456], F32,
                                                    tag="ev", name="ev")
                                        for _ in grp]}
                               if CONV_SCHEME == "ilvp" and RES_ON_DVE
                               else {}),
                        )
                for r0, nr in [] if CONV_SCHEME in ("ilvp", "pair", "bdg", "dr") else _row_chunks(c1lo, c1hi):
                    _conv_bankset(
                        nc,
                        ps1.tile(PS_SHAPE, F32, tag="ps1", name="pt1"),
                        w1, id1, s1_t,
                        soff=lambda t, _r=r0: 2
                        + (xloc(_r) + t // 3 - 1) * WP
                        + (t % 3 - 1),
                        res_t=x_t,
                        roff=2 + xloc(r0) * WP,
                        n=nr * WP,
                        asc=as1,
                        bias_mid=v["b12"],
                        alpha=v["a1"],
                        out_ap=p1_t[
                            :, 2 + ploc(r0) * WP : 2 + (ploc(r0) + nr) * WP
                        ],
                        ev=evpool.tile([128, 456], F32, tag="ev", name="ev"),
                    )

                # -- s2 = sign(p1 + b13 + b21), zero padding ------------
                if CONV_SCHEME == "dr":
                    SEG2 = -(-(P_LEN + 4) // 16) * 16
                    s2p_t = s2pool.tile([128, 2, SEG2], FP8, tag="s2")
                    s2_t = s2p_t[:, 0, :]
                else:
                    s2_t = s2pool.tile([128, P_LEN + 4], BF16, tag="s2")
                s2_r = s2_t[:, 2 : 2 + P_LEN].rearrange(
                    "p (r c) -> p r c", c=WP
                )
                nc.scalar.activation(
                    out=s2_t[:, 2 + ploc(c1lo) * WP : 2 + ploc(c1hi) * WP],
                    in_=p1_t[:, 2 + ploc(c1lo) * WP : 2 + ploc(c1hi) * WP].bitcast(F32),
                    func=mybir.ActivationFunctionType.Sign,
                    bias=b31,
                )
                nc.gpsimd.memset(s2_r[:, :, 0:1], 0.0)
                nc.gpsimd.memset(s2_r[:, :, WP - 1 : WP], 0.0)
                nc.gpsimd.memset(s2_t[:, 0:2], 0.0)
                nc.gpsimd.memset(s2_t[:, 2 + P_LEN :], 0.0)
                if ploc(c1lo) > 0:
                    nc.gpsimd.memset(s2_t[:, 2 : 2 + ploc(c1lo) * WP], 0.0)
                if ploc(c1hi) < P_ROWS:
                    nc.gpsimd.memset(
                        s2_t[:, 2 + ploc(c1hi) * WP : 2 + P_LEN], 0.0
                    )
                if CONV_SCHEME == "dr":
                    nc.vector.tensor_copy(
                        out=s2p_t[:, 1, 0 : SEG2 - 1],
                        in_=s2p_t[:, 0, 1:SEG2],
                    )
                    nc.gpsimd.memset(s2p_t[:, 1, SEG2 - 1 : SEG2], 0.0)

                # -- conv2 + fused chain --------------------------------
                p2_t = p2pool.tile([128, R * WP], F32, tag="p2")
                if CONV_SCHEME == "dr":
                    chunks = list(_row_chunks(h0, h0 + R))
                    for i in range(0, len(chunks), 2):
                        grp = chunks[i : i + 2]
                        _conv_group_dr(
                            nc,
                            [ps2.tile(PS_SHAPE, F32, tag="ps2", name="pt2")
                             for _ in grp],
                            w2[0], w2[1], id2, s2p_t,
                            soffs=[
                                lambda t, _r=r0: 2
                                + (ploc(_r) + t // 3 - 1) * WP + (t % 3 - 1)
                                for r0, _ in grp
                            ],
                            res_t=p1_t,
                            roffs=[2 + ploc(r0) * WP for r0, _ in grp],
                            ns=[nr * WP for _, nr in grp],
                            asc=as2, bias_mid=b32, alpha=v["a2"],
                            out_aps=[
                                p2_t[:, (r0 - h0) * WP : (r0 - h0 + nr) * WP]
                                for r0, nr in grp
                            ],
                        )
                if CONV_SCHEME == "pair":
                    sp2_t = _sp_build(nc, sp2pool, s2_t, P_LEN + 4, "sp2")
                    chunks = list(_row_chunks(h0, h0 + R))
                    for i in range(0, len(chunks), 2):
                        grp = chunks[i : i + 2]
                        _conv_group_pair(
                            nc,
                            [ps2.tile(PS_SHAPE, F32, tag="ps2", name="pt2")
                             for _ in grp],
                            w2, id2, s2_t, sp2_t, P_LEN + 4,
                            soffs=[
                                lambda t, _r=r0: 2
                                + (ploc(_r) + t // 3 - 1) * WP + (t % 3 - 1)
                                for r0, _ in grp
                            ],
                            res_t=p1_t,
                            roffs=[2 + ploc(r0) * WP for r0, _ in grp],
                            ns=[nr * WP for _, nr in grp],
                            asc=as2, bias_mid=b32, alpha=v["a2"],
                            out_aps=[
                                p2_t[:, (r0 - h0) * WP : (r0 - h0 + nr) * WP]
                                for r0, nr in grp
                            ],
                        )
                if CONV_SCHEME in ("ilvp", "bdg"):
                    chunks = list(_row_chunks(h0, h0 + R))
                    grp_fn = (_conv_group if CONV_SCHEME == "ilvp"
                              else _conv_group_bdg)
                    for i in range(0, len(chunks), 2):
                        grp = chunks[i : i + 2]
                        grp_fn(
                            nc,
                            [ps2.tile(PS_SHAPE, F32, tag="ps2", name="pt2")
                             for _ in grp],
                            w2, id2, s2_t,
                            soffs=[
                                lambda t, _r=r0: 2
                                + (ploc(_r) + t // 3 - 1) * WP + (t % 3 - 1)
                                for r0, _ in grp
                            ],
                            res_t=p1_t,
                            roffs=[2 + ploc(r0) * WP for r0, _ in grp],
                            ns=[nr * WP for _, nr in grp],
                            asc=as2, bias_mid=b32, alpha=v["a2"],
                            out_aps=[
                                p2_t[:, (r0 - h0) * WP : (r0 - h0 + nr) * WP]
                                for r0, nr in grp
                            ],
                            **({"evs": [evpool.tile([128, 456], F32,
                                                    tag="ev", name="ev")
                                        for _ in grp]}
                               if CONV_SCHEME == "ilvp" and RES_ON_DVE
                               else {}),
                        )
                for r0, nr in [] if CONV_SCHEME in ("ilvp", "pair", "bdg", "dr") else _row_chunks(h0, h0 + R):
                    _conv_bankset(
                        nc,
                        ps2.tile(PS_SHAPE, F32, tag="ps2", name="pt2"),
                        w2, id2, s2_t,
                        soff=lambda t, _r=r0: 2
                        + (ploc(_r) + t // 3 - 1) * WP
                        + (t % 3 - 1),
                        res_t=p1_t,
                        roff=2 + ploc(r0) * WP,
                        n=nr * WP,
                        asc=as2,
                        bias_mid=b32,
                        alpha=v["a2"],
                        out_ap=p2_t[
                            :, (r0 - h0) * WP : (r0 - h0 + nr) * WP
                        ],
                        ev=None if CONV_SCHEME in ("orig", "ilv", "ilvp")
                        else evpool.tile([128, 456], F32, tag="ev", name="ev"),
                    )

                # -- out2 = p2 + b23, store -----------------------------
                nc.vector.tensor_scalar_add(p2_t, p2_t, v["b23"])
                p2_r = p2_t.rearrange("p (r c) -> p r c", c=WP)
                for j in range(2):
                    nc.sync.dma_start(
                        out=out_d[imgs[j], :, h0 : h0 + R, :],
                        in_=p2_r[64 * j : 64 * j + 64, :, 1 : 1 + W],
                    )


_NC_CACHE = {}


def _get_program(bl=BL):
    if bl not in _NC_CACHE:
        _NC_CACHE[bl] = build_program(bl)
    return _NC_CACHE[bl]


def make_in_maps(inputs):
    x = np.ascontiguousarray(np.asarray(inputs["x"], dtype=np.float32))
    shared = {
        "w3": np.ascontiguousarray(np.asarray(inputs["w3"], np.float32)),
        "w_pw": np.ascontiguousarray(np.asarray(inputs["w_pw"], np.float32)),
    }
    for n in WVEC_NAMES:
        shared[n] = np.ascontiguousarray(np.asarray(inputs[n], np.float32))
    return [{"x": x[i * BL : (i + 1) * BL], **shared} for i in range(NCORES)]


def run(inputs, trace=False, **kwargs):
    nc = _get_program(BL)
    res = run_bass_kernel_spmd(
        nc, make_in_maps(inputs), core_ids=list(range(NCORES)), trace=trace,
        **kwargs,
    )
    out = np.concatenate([r["out"] for r in res.results], axis=0)
    return out, res


def kernel(**inputs):
    return run(inputs)[0]


def bench_loop(inputs, loops=(1, 9), reps=4):
    """Device-side timing: build the kernel with an on-device For_i repeat
    of the whole computation; slope between two loop counts gives per-
    iteration device time, independent of dispatch overhead."""
    import time

    in_maps = make_in_maps(inputs)
    times = {}
    for L in loops:
        nc = build_program(BL, loop_n=L)
        ts = []
        for _ in range(reps):
            t0 = time.perf_counter()
            run_bass_kernel_spmd(nc, in_maps, core_ids=list(range(NCORES)))
            ts.append(time.perf_counter() - t0)
        times[L] = min(ts)
        print(f"  loop_n={L}: {[f'{t*1e3:.1f}ms' for t in ts]}")
    l0, l1 = loops
    per_iter = (times[l1] - times[l0]) / (l1 - l0)
    return {"per_iter_s": per_iter, "times": times}


def bench(inputs, iters=20, nc=None):
    """Steady-state wall-clock benchmark: sharded jit without donation,
    device-resident inputs, async dispatch of `iters` executions."""
    import time
    import jax
    from jax.sharding import Mesh, PartitionSpec, NamedSharding
    from jax.experimental.shard_map import shard_map
    from concourse import bass2jax as b2j

    b2j.install_neuronx_cc_hook()
    if nc is None:
        nc = _get_program(BL)
    in_maps = make_in_maps(inputs)

    in_names, out_names, out_avals = [], [], []
    for alloc in nc.m.functions[0].allocations:
        if not isinstance(mybir.MemoryLocationSet, type) or not isinstance(
            alloc, mybir.MemoryLocationSet
        ):
            continue
        name = alloc.memorylocations[0].name
        if alloc.kind == "ExternalInput":
            if nc.partition_id_tensor and name == nc.partition_id_tensor.name:
                continue
            in_names.append(name)
        elif alloc.kind == "ExternalOutput":
            out_names.append(name)
            out_avals.append(
                jax.core.ShapedArray(
                    tuple(alloc.tensor_shape), mybir.dt.np(alloc.dtype)
                )
            )
    n_params = len(in_names)
    all_names = in_names + out_names
    if nc.partition_id_tensor:
        all_names = all_names + [nc.partition_id_tensor.name]

    def _body(*args):
        operands = list(args)
        if nc.partition_id_tensor:
            operands.append(b2j.partition_id_tensor())
        outs = b2j._bass_exec_p.bind(
            *operands,
            out_avals=tuple(out_avals),
            in_names=tuple(all_names),
            out_names=tuple(out_names),
            lowering_input_output_aliases=(),
            sim_require_finite=True,
            sim_require_nnan=True,
            nc=nc,
        )
        return tuple(outs)

    devices = jax.devices()[:NCORES]
    mesh = Mesh(np.asarray(devices), ("core",))
    nin = n_params + len(out_names)
    f = jax.jit(
        shard_map(
            _body,
            mesh=mesh,
            in_specs=(PartitionSpec("core"),) * nin,
            out_specs=(PartitionSpec("core"),) * len(out_names),
            check_rep=False,
        ),
        keep_unused=True,
    )
    sh = NamedSharding(mesh, PartitionSpec("core"))
    concat_in = [
        jax.device_put(np.concatenate([m[n] for m in in_maps], axis=0), sh)
        for n in in_names
    ]
    zeros = [
        jax.device_put(
            np.zeros((NCORES * a.shape[0], *a.shape[1:]), a.dtype), sh
        )
        for a in out_avals
    ]

    r = f(*concat_in, *zeros)  # warm-up / compile
    jax.block_until_ready(r)

    ts = []
    for _ in range(max(iters, 8)):
        t0 = time.perf_counter()
        r = f(*concat_in, *zeros)
        jax.block_until_ready(r)
        ts.append(time.perf_counter() - t0)
    return {"single_s": min(ts), "all": ts}


def bench_device(inputs, loops=(4, 16), calls=8):
    """Per-iteration device time via on-device For_i repetition: time
    single dispatches of programs that loop the computation loops[i]
    times; the slope cancels dispatch/transfer overhead."""
    res = {}
    for L in loops:
        nc = build_program(BL, loop_n=L)
        res[L] = bench(inputs, iters=calls, nc=nc)["single_s"]
        print(f"  loop_n={L}: best single call {res[L] * 1e3:.2f} ms")
    l0, l1 = loops
    per_iter = (res[l1] - res[l0]) / (l1 - l0)
    return {"per_iter_s": per_iter, "times": res}


if __name__ == "__main__":
    rng = np.random.default_rng(0)
    ins = {"x": rng.standard_normal((B, C, H, W)).astype(np.float32)}
    for n in ["w3", "w_pw"]:
        ins[n] = ((rng.random((C, C, 3, 3)) - 0.5) * 0.002).astype(np.float32)
    for n in WVEC_NAMES:
        ins[n] = (rng.standard_normal(C) * 0.01).astype(np.float32)
    out = kernel(**ins)
    print(out.shape, out.dtype)

